# revision 4
# baseline (speedup 1.0000x reference)
"""Trainium2 Bass kernel for nn_DecoderBlock (vanilla RNN decoder).

h_t = tanh(x_t @ U + h_{t-1} @ V + b), scanned over 512 timesteps.
Returns (encoder_inputs, decoder_outputs) matching the reference.

Strategy (8 NeuronCores, data-parallel over batch, 32 rows/core):
- State kept TRANSPOSED (hidden on partitions) as bf16: one SBUF ring tile
  [128, 2*32] per step: col = half*32*... see layout notes below.
- Z = x@U + b precomputed per 16-step chunk straight into PSUM via matmuls
  (rank-1 ones x b matmul broadcasts the bias and clears the bank; U-matmuls
  accumulate). The per-step critical path is then only 4 V-matmuls
  accumulating into the Z bank + one Tanh activation spanning both banks.
- All wire dtypes bf16 (fp32 PSUM accumulation); host pre-transposes inputs
  and un-permutes outputs (pure layout work, no FLOPs on host).
"""

import os
import numpy as np
import ml_dtypes

BZ, SEQ, IN_SZ, HID = 256, 512, 256, 256
NCORES = 8
BC = BZ // NCORES          # 32 batch rows per core
TCH = 16                   # timesteps per Z-chunk (one PSUM bank pair)
NCH = SEQ // TCH           # 32 chunks
OCH = 64                   # timesteps per output ring/DMA window
BF16 = ml_dtypes.bfloat16

LAST_EXEC_NS = None

_built = {}


def _build_module():
    """Build + compile the Bass module (once per process)."""
    from contextlib import ExitStack
    import concourse.bass as bass
    import concourse.tile as tile
    from concourse import bacc, mybir

    dt = mybir.dt
    nc = bacc.Bacc(
        "TRN2",
        debug=False,
        enable_asserts=False,
        num_devices=NCORES,
    )

    # DRAM I/O (per-core shard layouts, all bf16):
    #   xT  [2, 128, SEQ, BC]: xT[kh, k, t, b] = x[b, t, 128*kh + k]
    #   V   [256, 256] (as stored; lhsT tiles are V[kh*128:, jh*128:])
    #   U   [256, 256]
    #   bvec[1, 256]
    #   ones[1, TCH*BC]
    #   h0T [2, 128, BC]: h0T[h, j, b] = h0[b, 128*h + j]
    #   out [2, 128, SEQ, BC]: out[h, j, t, b] = h_t[b, 128*h + j]
    xT = nc.dram_tensor("xT", [2, 128, SEQ, BC], dt.bfloat16, kind="ExternalInput")
    V_ = nc.dram_tensor("V", [HID, HID], dt.bfloat16, kind="ExternalInput")
    U_ = nc.dram_tensor("U", [IN_SZ, HID], dt.bfloat16, kind="ExternalInput")
    bv = nc.dram_tensor("bvec", [1, HID], dt.bfloat16, kind="ExternalInput")
    on = nc.dram_tensor("ones", [1, TCH * BC], dt.bfloat16, kind="ExternalInput")
    h0 = nc.dram_tensor("h0T", [2, 128, BC], dt.bfloat16, kind="ExternalInput")
    out = nc.dram_tensor("out", [2, 128, SEQ, BC], dt.bfloat16, kind="ExternalOutput")

    TANH = mybir.ActivationFunctionType.Tanh

    with tile.TileContext(nc) as tc, ExitStack() as ctx:
        consts = ctx.enter_context(tc.tile_pool(name="consts", bufs=1))
        xpool = ctx.enter_context(tc.tile_pool(name="xin", bufs=3))
        zpool = ctx.enter_context(tc.tile_pool(name="zps", bufs=3, space="PSUM"))
        ringp = ctx.enter_context(tc.tile_pool(name="ring", bufs=2))

        # ---- constants ----
        # Vt cols: kh*256 + j  (Vt[:, kh*256+j][p] = V[128*kh + p, j])
        Vt = consts.tile([128, 2 * HID], dt.bfloat16)
        nc.sync.dma_start(out=Vt[:, 0:HID], in_=V_.ap()[0:128, :])
        nc.sync.dma_start(out=Vt[:, HID : 2 * HID], in_=V_.ap()[128:256, :])
        Ut = consts.tile([128, 2 * HID], dt.bfloat16)
        nc.sync.dma_start(out=Ut[:, 0:HID], in_=U_.ap()[0:128, :])
        nc.sync.dma_start(out=Ut[:, HID : 2 * HID], in_=U_.ap()[128:256, :])
        bt = consts.tile([1, HID], dt.bfloat16)
        nc.sync.dma_start(out=bt[:, :], in_=bv.ap()[:, :])
        ot = consts.tile([1, TCH * BC], dt.bfloat16)
        nc.sync.dma_start(out=ot[:, :], in_=on.ap()[:, :])
        # initial state, col = h*BC + b
        h0t = consts.tile([128, 2 * BC], dt.bfloat16)
        nc.sync.dma_start(out=h0t[:, 0:BC], in_=h0.ap()[0, :, :])
        nc.sync.dma_start(out=h0t[:, BC : 2 * BC], in_=h0.ap()[1, :, :])

        ZCOLS = TCH * BC  # 512 fp32 = one PSUM bank per half

        x_tiles = {}
        z_tiles = {}

        def emit_x_dma(c):
            if c >= NCH or c in x_tiles:
                return
            xt = xpool.tile([128, 2 * ZCOLS], dt.bfloat16, name=f"xt{c}", tag="xt")
            x_tiles[c] = xt
            dst = xt[:, :].rearrange("p (k t b) -> p k t b", k=2, t=TCH)
            src = xT.ap().rearrange("k p t b -> p k t b")[
                :, :, c * TCH : (c + 1) * TCH, :
            ]
            nc.sync.dma_start(out=dst, in_=src)

        def emit_z_mms(c, part):
            """Emit Z matmuls for chunk c; part 0 or 1 selects output half."""
            if c >= NCH:
                return
            if c not in z_tiles:
                z_tiles[c] = zpool.tile([128, 2 * ZCOLS], dt.float32, name=f"zt{c}", tag="zt")
            zt = z_tiles[c]
            xt = x_tiles[c]
            h = part
            zslice = zt[:, h * ZCOLS : (h + 1) * ZCOLS]
            # bias broadcast (clears the bank: first matmul of the group)
            nc.tensor.matmul(
                zslice,
                bt[0:1, h * 128 : (h + 1) * 128],
                ot[0:1, :],
                start=True,
                stop=False,
                skip_group_check=True,
            )
            for kh in (0, 1):
                nc.tensor.matmul(
                    zslice,
                    Ut[:, kh * HID + h * 128 : kh * HID + (h + 1) * 128],
                    xt[:, kh * ZCOLS : (kh + 1) * ZCOLS],
                    start=False,
                    stop=False,
                    skip_group_check=True,
                )

        # ---- prologue: fill pipeline ----
        emit_x_dma(0)
        emit_x_dma(1)
        for h in (0, 1):
            emit_z_mms(0, h)
        for h in (0, 1):
            emit_z_mms(1, h)
        emit_x_dma(2)

        ring_cur = None
        ring_prev = None

        def rhs_ap(t, kh):
            """State h_{t} slice for contraction half kh."""
            if t < 0:
                return h0t[:, kh * BC : (kh + 1) * BC]
            tile_ = ring_cur if (t // OCH) == (t_now // OCH) else ring_prev
            tl = t % OCH
            base = kh * OCH * BC + tl * BC
            return tile_[:, base : base + BC]

        for c in range(NCH):
            zt = z_tiles[c]
            for tl in range(TCH):
                t_now = c * TCH + tl
                if t_now % OCH == 0:
                    ring_prev = ring_cur
                    ring_cur = ringp.tile([128, OCH * 2 * BC], dt.bfloat16, name=f"ring{t_now // OCH}", tag="ring")
                # 4 V-matmuls accumulate on top of Z in PSUM
                last = None
                for h in (0, 1):
                    zcol = h * ZCOLS + tl * BC
                    for kh in (0, 1):
                        last = nc.tensor.matmul(
                            zt[:, zcol : zcol + BC],
                            Vt[:, kh * HID + h * 128 : kh * HID + (h + 1) * 128],
                            rhs_ap(t_now - 1, kh),
                            start=False,
                            stop=(h == 1 and kh == 1 and tl == TCH - 1),
                            skip_group_check=True,
                        )
                # tanh over both halves (strided AP across the 2 banks)
                zin = zt[:, :].rearrange("p (h t b) -> p h t b", h=2, t=TCH)[
                    :, :, tl : tl + 1, :
                ]
                tloc = t_now % OCH
                hout = ring_cur[:, :].rearrange(
                    "p (k t b) -> p k t b", k=2, t=OCH
                )[:, :, tloc : tloc + 1, :]
                nc.scalar.activation(hout, zin, TANH)

                # interleave future work into PE idle windows
                if tl == 0:
                    emit_x_dma(c + 3)
                if tl in (2, 5):
                    emit_z_mms(c + 2, 0)
                elif tl in (8, 11):
                    emit_z_mms(c + 2, 1)

                # output DMA at end of each 64-step window
                if t_now % OCH == OCH - 1:
                    w = t_now // OCH
                    src = ring_cur[:, :].rearrange(
                        "p (k t b) -> p k t b", k=2, t=OCH
                    )
                    dst = out.ap().rearrange("h j t b -> j h t b")[
                        :, :, w * OCH : (w + 1) * OCH, :
                    ]
                    nc.sync.dma_start(out=dst, in_=src)

    nc.compile()
    return nc


def _get_module():
    if "nc" not in _built:
        _built["nc"] = _build_module()
    return _built["nc"]


def _prep_in_maps(encoder_inputs, decoder_inputs, U_i, V_i, b_i):
    enc = np.asarray(encoder_inputs, dtype=np.float32)
    dec = np.asarray(decoder_inputs, dtype=np.float32)
    U = np.asarray(U_i, dtype=np.float32).astype(BF16)
    V = np.asarray(V_i, dtype=np.float32).astype(BF16)
    b = np.asarray(b_i, dtype=np.float32).reshape(1, HID).astype(BF16)
    ones = np.ones((1, TCH * BC), dtype=BF16)
    in_maps = []
    for c in range(NCORES):
        xs = dec[c * BC : (c + 1) * BC]                      # [BC, SEQ, IN]
        xTc = np.ascontiguousarray(xs.transpose(2, 1, 0)).astype(BF16)
        xTc = xTc.reshape(2, 128, SEQ, BC)
        h0c = enc[c * BC : (c + 1) * BC]                     # [BC, HID]
        h0T = np.ascontiguousarray(h0c.T).astype(BF16).reshape(2, 128, BC)
        in_maps.append(
            {"xT": xTc, "V": V, "U": U, "bvec": b, "ones": ones, "h0T": h0T}
        )
    return in_maps


def kernel(encoder_inputs, decoder_inputs, U_i, V_i, b_i):
    global LAST_EXEC_NS
    from concourse import bass_utils

    nc = _get_module()
    in_maps = _prep_in_maps(encoder_inputs, decoder_inputs, U_i, V_i, b_i)
    trace = os.environ.get("KERNEL_TRACE", "0") == "1"
    res = bass_utils.run_bass_kernel_spmd(
        nc, in_maps, core_ids=list(range(NCORES)), trace=trace
    )
    LAST_EXEC_NS = res.exec_time_ns

    outs = []
    for c in range(NCORES):
        o = np.asarray(res.results[c]["out"]).astype(np.float32)  # [2,128,SEQ,BC]
        o = o.reshape(HID, SEQ, BC).transpose(2, 1, 0)            # [BC, SEQ, HID]
        outs.append(o)
    dec_out = np.ascontiguousarray(np.concatenate(outs, axis=0))
    enc = np.asarray(encoder_inputs, dtype=np.float32)
    return (enc, dec_out)


# revision 17
# speedup vs baseline: 236.6737x; 236.6737x over previous
"""Trainium2 Bass kernel for nn_DecoderBlock (vanilla RNN decoder).

h_t = tanh(x_t @ U + h_{t-1} @ V + b), scanned over 512 timesteps.
Returns (encoder_inputs, decoder_outputs) matching the reference.

Strategy (8 NeuronCores, data-parallel over batch, 32 rows/core):
- State kept TRANSPOSED (hidden on partitions) as bf16 in an SBUF ring.
- Z = x@U + b precomputed per TCH-step chunk straight into PSUM via matmuls
  (rank-1 ones x b matmul broadcasts the bias and clears the bank; U-matmuls
  accumulate). The per-step critical path is then only 4 V-matmuls
  accumulating onto Z in place + one Tanh activation.
- S independent half-batch chains per core overlap one chain's activation
  latency with the other's matmuls. Every chain owns its own PSUM tiles and
  SBUF ring tiles: dependency tracking uses AP bounding boxes, so sharing a
  tile across chains would serialize them.
- All wire dtypes bf16 (fp32 PSUM accumulation); host pre-transposes inputs
  and un-permutes outputs (pure layout work, no FLOPs on host).
"""

import os
import numpy as np
import ml_dtypes

BZ, SEQ, IN_SZ, HID = 256, 512, 256, 256
NCORES = 8
BC = BZ // NCORES          # 32 batch rows per core
S = 2                      # independent chains per core
CB = BC // S               # batch rows per chain
TCH = 512 // CB            # timesteps per Z-chunk (fills one PSUM bank/region)
NCH = SEQ // TCH           # chunks
OCH = 64                   # timesteps per output ring/DMA window
BF16 = ml_dtypes.bfloat16

LAST_EXEC_NS = None

_built = {}


def _build_module():
    """Build + compile the Bass module (once per process)."""
    from contextlib import ExitStack
    import concourse.bass as bass
    import concourse.tile as tile
    from concourse import bacc, mybir

    dt = mybir.dt
    nc = bacc.Bacc(
        "TRN2",
        debug=False,
        enable_asserts=False,
        num_devices=NCORES,
    )

    # DRAM I/O (per-core shard layouts, all bf16):
    #   xT  [2, 128, SEQ, BC]: xT[kh, k, t, b] = x[b, t, 128*kh + k]
    #   V,U [256, 256] (as stored; lhsT tiles are W[kh*128:, jh*128:])
    #   bvec[1, 256]; ones[1, 512]
    #   h0T [2, 128, BC]: h0T[h, j, b] = h0[b, 128*h + j]
    #   out [S, 2, 128, SEQ, CB]: out[s, h, j, t, b] = h_t[s*CB + b, 128*h + j]
    xT = nc.dram_tensor("xT", [2, 128, SEQ, BC], dt.bfloat16, kind="ExternalInput")
    xL = nc.dram_tensor("xTlo", [2, 128, SEQ, BC], dt.bfloat16, kind="ExternalInput")
    V_ = nc.dram_tensor("V", [HID, HID], dt.bfloat16, kind="ExternalInput")
    U_ = nc.dram_tensor("U", [IN_SZ, HID], dt.bfloat16, kind="ExternalInput")
    Ul = nc.dram_tensor("Ulo", [IN_SZ, HID], dt.bfloat16, kind="ExternalInput")
    bv = nc.dram_tensor("bvec", [1, HID], dt.bfloat16, kind="ExternalInput")
    on = nc.dram_tensor("ones", [1, 512], dt.bfloat16, kind="ExternalInput")
    h0 = nc.dram_tensor("h0T", [2, 128, BC], dt.bfloat16, kind="ExternalInput")
    out = nc.dram_tensor(
        "out", [S, 2, 128, SEQ, CB], dt.bfloat16, kind="ExternalOutput"
    )

    TANH = mybir.ActivationFunctionType.Tanh
    ZREG = TCH * CB            # 512 fp32 = one PSUM bank per half
    ZBUFS = 4 if S == 2 else 3

    with tile.TileContext(nc) as tc, ExitStack() as ctx:
        consts = ctx.enter_context(tc.tile_pool(name="consts", bufs=1))
        xpool = ctx.enter_context(tc.tile_pool(name="xin", bufs=3))
        zpool = ctx.enter_context(tc.tile_pool(name="zps", bufs=ZBUFS, space="PSUM"))
        ringp = ctx.enter_context(tc.tile_pool(name="ring", bufs=2 * S))

        # ---- constants (one DMA per tensor via k-split rearrange) ----
        # Vt cols: kh*256 + j  (Vt[:, kh*256+j][p] = V[128*kh + p, j])
        Vt = consts.tile([128, 2 * HID], dt.bfloat16)
        nc.sync.dma_start(
            out=Vt[:, :].rearrange("p (k j) -> p k j", k=2),
            in_=V_.ap().rearrange("(k p) j -> p k j", k=2),
        )
        Ut = consts.tile([128, 2 * HID], dt.bfloat16)
        nc.sync.dma_start(
            out=Ut[:, :].rearrange("p (k j) -> p k j", k=2),
            in_=U_.ap().rearrange("(k p) j -> p k j", k=2),
        )
        Ult = consts.tile([128, 2 * HID], dt.bfloat16)
        nc.sync.dma_start(
            out=Ult[:, :].rearrange("p (k j) -> p k j", k=2),
            in_=Ul.ap().rearrange("(k p) j -> p k j", k=2),
        )
        bt = consts.tile([1, HID], dt.bfloat16)
        nc.sync.dma_start(out=bt[:, :], in_=bv.ap()[:, :])
        ot = consts.tile([1, 512], dt.bfloat16)
        nc.sync.dma_start(out=ot[:, :], in_=on.ap()[:, :])
        # initial state, col = kh*BC + b
        h0t = consts.tile([128, 2 * BC], dt.bfloat16)
        nc.sync.dma_start(
            out=h0t[:, :].rearrange("p (k b) -> p k b", k=2),
            in_=h0.ap().rearrange("k p b -> p k b"),
        )

        x_tiles = {}
        z_tiles = {}        # (chunk, chain) -> psum tile [128, 2*ZREG]
        ring_tiles = {}     # (window, chain) -> sbuf tile [128, OCH*2*CB]

        def emit_x_dma(c):
            if c >= NCH or c in x_tiles:
                return
            xt = xpool.tile(
                [128, 2 * TCH * BC], dt.bfloat16, name=f"xt{c}", tag="xt"
            )
            xlt = xpool.tile(
                [128, 2 * TCH * BC], dt.bfloat16, name=f"xlt{c}", tag="xlt"
            )
            x_tiles[c] = (xt, xlt)
            for tile_, dram in ((xt, xT), (xlt, xL)):
                dst = tile_[:, :].rearrange("p (k t b) -> p k t b", k=2, t=TCH)
                src = dram.ap().rearrange("k p t b -> p k t b")[
                    :, :, c * TCH : (c + 1) * TCH, :
                ]
                nc.sync.dma_start(out=dst, in_=src)

        def _z_tile(c, chain):
            key = (c, chain)
            if key not in z_tiles:
                z_tiles[key] = zpool.tile(
                    [128, 2 * ZREG], dt.float32, name=f"zt{c}_{chain}", tag="zt"
                )
            return z_tiles[key]

        def z_work(c):
            """Thunks, one Z-matmul each, for chunk c (spread across steps).
            Order within a (chain, half) region is preserved by emission
            order; the bias matmul (start=True) clears the bank first."""
            if c >= NCH:
                return []
            thunks = []

            def mm(chain, h, lhsT_fn, rhs_fn, start):
                def _emit():
                    zt = _z_tile(c, chain)
                    zslice = zt[:, h * ZREG : (h + 1) * ZREG]
                    nc.tensor.matmul(
                        zslice,
                        lhsT_fn(),
                        rhs_fn(),
                        start=start,
                        stop=False,
                        skip_group_check=True,
                    )
                return _emit

            for chain in range(S):
                for h in (0, 1):
                    thunks.append(
                        mm(
                            chain, h,
                            lambda h=h: bt[0:1, h * 128 : (h + 1) * 128],
                            lambda: ot[0:1, 0:ZREG],
                            start=True,
                        )
                    )
                    for kh in (0, 1):
                        def xs_hi(kh=kh, chain=chain):
                            xt = x_tiles[c][0]
                            return xt[:, :].rearrange(
                                "p (k t b) -> p k t b", k=2, t=TCH
                            )[:, kh, :, chain * CB : (chain + 1) * CB]

                        def xs_lo(kh=kh, chain=chain):
                            xlt = x_tiles[c][1]
                            return xlt[:, :].rearrange(
                                "p (k t b) -> p k t b", k=2, t=TCH
                            )[:, kh, :, chain * CB : (chain + 1) * CB]

                        def u_hi(kh=kh, h=h):
                            return Ut[:, kh * HID + h * 128 : kh * HID + (h + 1) * 128]

                        def u_lo(kh=kh, h=h):
                            return Ult[:, kh * HID + h * 128 : kh * HID + (h + 1) * 128]

                        # split-bf16: U_hi*x_hi + U_hi*x_lo + U_lo*x_hi
                        thunks.append(mm(chain, h, u_hi, xs_hi, start=False))
                        thunks.append(mm(chain, h, u_hi, xs_lo, start=False))
                        thunks.append(mm(chain, h, u_lo, xs_hi, start=False))
            return thunks



        # ---- prologue: fill pipeline ----
        emit_x_dma(0)
        emit_x_dma(1)
        for th in z_work(0):
            th()

        def ring_view(w, chain):
            return ring_tiles[(w, chain)][:, :].rearrange(
                "p (k t b) -> p k t b", k=2, t=OCH
            )

        def rhs_ap(t, kh, chain):
            """State h_{t} slice [128, CB] for contraction half kh."""
            if t < 0:
                return h0t[:, kh * BC + chain * CB : kh * BC + (chain + 1) * CB]
            return ring_view(t // OCH, chain)[:, kh, t % OCH, :]

        pending_z = []

        for c in range(NCH):
            pending_z = z_work(c + 1)
            for tl in range(TCH):
                t_now = c * TCH + tl
                if t_now % OCH == 0:
                    w = t_now // OCH
                    for chain in range(S):
                        ring_tiles[(w, chain)] = ringp.tile(
                            [128, OCH * 2 * CB],
                            dt.bfloat16,
                            name=f"ring{w}_{chain}",
                            tag="ring",
                        )
                if tl == 0:
                    emit_x_dma(c + 2)
                for chain in range(S):
                    zt = z_tiles[(c, chain)]
                    # 4 V-matmuls accumulate on top of Z in PSUM
                    for h in (0, 1):
                        zcol = h * ZREG + tl * CB
                        for kh in (0, 1):
                            nc.tensor.matmul(
                                zt[:, zcol : zcol + CB],
                                Vt[:, kh * HID + h * 128 : kh * HID + (h + 1) * 128],
                                rhs_ap(t_now - 1, kh, chain),
                                start=False,
                                stop=(h == 1 and kh == 1 and tl == TCH - 1),
                                skip_group_check=True,
                            )
                    # tanh over both halves of this chain (AP across 2 banks)
                    zin = zt[:, :].rearrange("p (h t b) -> p h t b", h=2, t=TCH)[
                        :, :, tl, :
                    ]
                    hout = ring_view(t_now // OCH, chain)[:, :, t_now % OCH, :]
                    nc.scalar.activation(hout, zin, TANH)

                # interleave next chunk's Z work into PE idle windows,
                # one matmul per step so the in-order PE stream never
                # stalls the recurrence chain for long
                if tl >= 1 and pending_z:
                    pending_z.pop(0)()

                # output DMA at end of each OCH-step window
                if t_now % OCH == OCH - 1:
                    w = t_now // OCH
                    for chain in range(S):
                        src = ring_view(w, chain)
                        dst = out.ap()[chain].rearrange("h j t b -> j h t b")[
                            :, :, w * OCH : (w + 1) * OCH, :
                        ]
                        nc.sync.dma_start(out=dst, in_=src)

    nc.compile()
    return nc


def _get_module():
    if "nc" not in _built:
        _built["nc"] = _build_module()
    return _built["nc"]


def _prep_in_maps(encoder_inputs, decoder_inputs, U_i, V_i, b_i):
    enc = np.asarray(encoder_inputs, dtype=np.float32)
    dec = np.asarray(decoder_inputs, dtype=np.float32)
    U32 = np.asarray(U_i, dtype=np.float32)
    U = U32.astype(BF16)
    Ulo = (U32 - U.astype(np.float32)).astype(BF16)
    V = np.asarray(V_i, dtype=np.float32).astype(BF16)
    b = np.asarray(b_i, dtype=np.float32).reshape(1, HID).astype(BF16)
    ones = np.ones((1, 512), dtype=BF16)
    in_maps = []
    for c in range(NCORES):
        xs = dec[c * BC : (c + 1) * BC]                      # [BC, SEQ, IN]
        xT32 = np.ascontiguousarray(xs.transpose(2, 1, 0))   # [IN, SEQ, BC]
        xTc = xT32.astype(BF16)
        xTlo = (xT32 - xTc.astype(np.float32)).astype(BF16)
        xTc = xTc.reshape(2, 128, SEQ, BC)
        xTlo = xTlo.reshape(2, 128, SEQ, BC)
        h0c = enc[c * BC : (c + 1) * BC]                     # [BC, HID]
        h0T = np.ascontiguousarray(h0c.T).astype(BF16).reshape(2, 128, BC)
        in_maps.append(
            {
                "xT": xTc,
                "xTlo": xTlo,
                "V": V,
                "U": U,
                "Ulo": Ulo,
                "bvec": b,
                "ones": ones,
                "h0T": h0T,
            }
        )
    return in_maps


def kernel(encoder_inputs, decoder_inputs, U_i, V_i, b_i):
    global LAST_EXEC_NS
    from concourse import bass_utils

    nc = _get_module()
    in_maps = _prep_in_maps(encoder_inputs, decoder_inputs, U_i, V_i, b_i)
    trace = os.environ.get("KERNEL_TRACE", "0") == "1"
    res = bass_utils.run_bass_kernel_spmd(
        nc, in_maps, core_ids=list(range(NCORES)), trace=trace
    )
    LAST_EXEC_NS = res.exec_time_ns

    outs = []
    for c in range(NCORES):
        o = np.asarray(res.results[c]["out"]).astype(np.float32)  # [S,2,128,SEQ,CB]
        o = o.reshape(S, HID, SEQ, CB)          # j = 128*h + j'
        o = o.transpose(0, 3, 2, 1)             # [S, CB, SEQ, HID]
        outs.append(o.reshape(BC, SEQ, HID))    # b = chain*CB + b'
    dec_out = np.ascontiguousarray(np.concatenate(outs, axis=0))
    enc = np.asarray(encoder_inputs, dtype=np.float32)
    return (enc, dec_out)


# revision 27
# speedup vs baseline: 237.0424x; 1.0016x over previous
"""Trainium2 Bass kernel for nn_DecoderBlock (vanilla RNN decoder).

h_t = tanh(x_t @ U + h_{t-1} @ V + b), scanned over 512 timesteps.
Returns (encoder_inputs, decoder_outputs) matching the reference.

Strategy (8 NeuronCores, data-parallel over batch, 32 rows/core):
- State kept TRANSPOSED (hidden on partitions) as bf16 in an SBUF ring.
- Z = x@U + b precomputed per TCH-step chunk straight into PSUM via matmuls
  (rank-1 ones x b matmul broadcasts the bias and clears the bank; U-matmuls
  accumulate). The per-step critical path is then only 4 V-matmuls
  accumulating onto Z in place + one Tanh activation.
- S independent half-batch chains per core overlap one chain's activation
  latency with the other's matmuls. Every chain owns its own PSUM tiles and
  SBUF ring tiles: dependency tracking uses AP bounding boxes, so sharing a
  tile across chains would serialize them.
- All wire dtypes bf16 (fp32 PSUM accumulation); host pre-transposes inputs
  and un-permutes outputs (pure layout work, no FLOPs on host).
"""

import os
import numpy as np
import ml_dtypes

BZ, SEQ, IN_SZ, HID = 256, 512, 256, 256
NCORES = 8
BC = BZ // NCORES          # 32 batch rows per core
S = 2                      # independent chains per core
CB = BC // S               # batch rows per chain
TCH = 512 // CB            # timesteps per Z-chunk (fills one PSUM bank/region)
NCH = SEQ // TCH           # chunks
OCH = 64                   # timesteps per output ring/DMA window
BF16 = ml_dtypes.bfloat16

LAST_EXEC_NS = None

_built = {}


def _build_module():
    """Build + compile the Bass module (once per process)."""
    from contextlib import ExitStack
    import concourse.bass as bass
    import concourse.tile as tile
    from concourse import bacc, mybir

    dt = mybir.dt
    nc = bacc.Bacc(
        "TRN2",
        debug=False,
        enable_asserts=False,
        num_devices=NCORES,
    )

    # DRAM I/O (per-core shard layouts, all bf16):
    #   xT  [2, 128, SEQ, BC]: xT[kh, k, t, b] = x[b, t, 128*kh + k]
    #   V,U [256, 256] (as stored; lhsT tiles are W[kh*128:, jh*128:])
    #   bvec[1, 256]; ones[1, 512]
    #   h0T [2, 128, BC]: h0T[h, j, b] = h0[b, 128*h + j]
    #   out [S, 2, 128, SEQ, CB]: out[s, h, j, t, b] = h_t[s*CB + b, 128*h + j]
    xT = nc.dram_tensor("xT", [2, 128, SEQ, BC], dt.bfloat16, kind="ExternalInput")
    xL = nc.dram_tensor("xTlo", [2, 128, SEQ, BC], dt.bfloat16, kind="ExternalInput")
    V_ = nc.dram_tensor("V", [HID, HID], dt.bfloat16, kind="ExternalInput")
    U_ = nc.dram_tensor("U", [IN_SZ, HID], dt.bfloat16, kind="ExternalInput")
    Ul = nc.dram_tensor("Ulo", [IN_SZ, HID], dt.bfloat16, kind="ExternalInput")
    bv = nc.dram_tensor("bvec", [1, HID], dt.bfloat16, kind="ExternalInput")
    on = nc.dram_tensor("ones", [1, 512], dt.bfloat16, kind="ExternalInput")
    h0 = nc.dram_tensor("h0T", [2, 128, BC], dt.bfloat16, kind="ExternalInput")
    out = nc.dram_tensor(
        "out", [S, 2, 128, SEQ, CB], dt.bfloat16, kind="ExternalOutput"
    )

    TANH = mybir.ActivationFunctionType.Tanh
    ZREG = TCH * CB            # 512 fp32 = one PSUM bank per half
    ZBUFS = 4 if S == 2 else 3

    with tile.TileContext(nc) as tc, ExitStack() as ctx:
        consts = ctx.enter_context(tc.tile_pool(name="consts", bufs=1))
        xpool = ctx.enter_context(tc.tile_pool(name="xin", bufs=3))
        zpool = ctx.enter_context(tc.tile_pool(name="zps", bufs=ZBUFS, space="PSUM"))
        ringp = ctx.enter_context(tc.tile_pool(name="ring", bufs=2 * S))

        # ---- constants (one DMA per tensor via k-split rearrange) ----
        Vt = consts.tile([128, 2 * HID], dt.bfloat16)
        Ut = consts.tile([128, 2 * HID], dt.bfloat16)
        Ult = consts.tile([128, 2 * HID], dt.bfloat16)
        bt = consts.tile([1, HID], dt.bfloat16)
        ot = consts.tile([1, 512], dt.bfloat16)
        h0t = consts.tile([128, 2 * BC], dt.bfloat16)

        def emit_const_dmas():
            nc.sync.dma_start(out=bt[:, :], in_=bv.ap()[:, :])
            nc.sync.dma_start(out=ot[:, :], in_=on.ap()[:, :])
            for tile_, dram in ((Ut, U_), (Ult, Ul), (Vt, V_)):
                nc.sync.dma_start(
                    out=tile_[:, :].rearrange("p (k j) -> p k j", k=2),
                    in_=dram.ap().rearrange("(k p) j -> p k j", k=2),
                )
            # initial state, col = kh*BC + b
            nc.sync.dma_start(
                out=h0t[:, :].rearrange("p (k b) -> p k b", k=2),
                in_=h0.ap().rearrange("k p b -> p k b"),
            )

        def w_slice(w, kh, jh):
            tile_ = (Vt, Ut, Ult)[w]
            return tile_[:, kh * HID + jh * 128 : kh * HID + (jh + 1) * 128]

        x_tiles = {}
        z_tiles = {}        # (chunk, chain) -> psum tile [128, 2*ZREG]
        ring_tiles = {}     # (window, chain) -> sbuf tile [128, OCH*2*CB]

        def emit_x_dma(c, eng=None):
            if c >= NCH or c in x_tiles:
                return
            # prologue x DMAs issue from the otherwise-idle ACT sequencer
            # (also a HWDGE engine) so they don't queue behind the const
            # DMAs on the SP sequencer
            eng = eng or nc.sync
            xt = xpool.tile(
                [128, 2 * TCH * BC], dt.bfloat16, name=f"xt{c}", tag="xt"
            )
            xlt = xpool.tile(
                [128, 2 * TCH * BC], dt.bfloat16, name=f"xlt{c}", tag="xlt"
            )
            x_tiles[c] = (xt, xlt)
            for tile_, dram in ((xt, xT), (xlt, xL)):
                dst = tile_[:, :].rearrange("p (k t b) -> p k t b", k=2, t=TCH)
                src = dram.ap().rearrange("k p t b -> p k t b")[
                    :, :, c * TCH : (c + 1) * TCH, :
                ]
                eng.dma_start(out=dst, in_=src)

        def _z_tile(c, chain):
            key = (c, chain)
            if key not in z_tiles:
                z_tiles[key] = zpool.tile(
                    [128, 2 * ZREG], dt.float32, name=f"zt{c}_{chain}", tag="zt"
                )
            return z_tiles[key]

        def z_work(c):
            """Thunks, one Z-matmul each, for chunk c (spread across steps).
            Order within a (chain, half) region is preserved by emission
            order; the bias matmul (start=True) clears the bank first."""
            if c >= NCH:
                return []
            thunks = []

            def mm(chain, h, lhsT_fn, rhs_fn, start):
                def _emit():
                    zt = _z_tile(c, chain)
                    zslice = zt[:, h * ZREG : (h + 1) * ZREG]
                    nc.tensor.matmul(
                        zslice,
                        lhsT_fn(),
                        rhs_fn(),
                        start=start,
                        stop=False,
                        skip_group_check=True,
                    )
                return _emit

            for chain in range(S):
                for h in (0, 1):
                    thunks.append(
                        mm(
                            chain, h,
                            lambda h=h: bt[0:1, h * 128 : (h + 1) * 128],
                            lambda: ot[0:1, 0:ZREG],
                            start=True,
                        )
                    )
                    for kh in (0, 1):
                        def xs_hi(kh=kh, chain=chain):
                            xt = x_tiles[c][0]
                            return xt[:, :].rearrange(
                                "p (k t b) -> p k t b", k=2, t=TCH
                            )[:, kh, :, chain * CB : (chain + 1) * CB]

                        def xs_lo(kh=kh, chain=chain):
                            xlt = x_tiles[c][1]
                            return xlt[:, :].rearrange(
                                "p (k t b) -> p k t b", k=2, t=TCH
                            )[:, kh, :, chain * CB : (chain + 1) * CB]

                        def u_hi(kh=kh, h=h):
                            return w_slice(1, kh, h)

                        def u_lo(kh=kh, h=h):
                            return w_slice(2, kh, h)

                        # split-bf16: U_hi*x_hi + U_hi*x_lo + U_lo*x_hi
                        thunks.append(mm(chain, h, u_hi, xs_hi, start=False))
                        thunks.append(mm(chain, h, u_hi, xs_lo, start=False))
                        thunks.append(mm(chain, h, u_lo, xs_hi, start=False))
            return thunks



        # ---- prologue: fill pipeline ----
        emit_const_dmas()
        emit_x_dma(0)
        emit_x_dma(1)
        # bank-clearing bias matmuls first: they only need bt/ot, so they
        # run while the x tiles are still in flight
        z0 = z_work(0)
        n_per_region = len(z0) // (2 * S)
        b_first = [z0[i] for i in range(0, len(z0), n_per_region)]
        rest = [th for i, th in enumerate(z0) if i % n_per_region != 0]
        for th in b_first + rest:
            th()

        def ring_view(w, chain):
            return ring_tiles[(w, chain)][:, :].rearrange(
                "p (k t b) -> p k t b", k=2, t=OCH
            )

        def rhs_ap(t, kh, chain):
            """State h_{t} slice [128, CB] for contraction half kh."""
            if t < 0:
                return h0t[:, kh * BC + chain * CB : kh * BC + (chain + 1) * CB]
            return ring_view(t // OCH, chain)[:, kh, t % OCH, :]

        pending_z = []

        for c in range(NCH):
            pending_z = z_work(c + 1)
            for tl in range(TCH):
                t_now = c * TCH + tl
                if t_now % OCH == 0:
                    w = t_now // OCH
                    for chain in range(S):
                        ring_tiles[(w, chain)] = ringp.tile(
                            [128, OCH * 2 * CB],
                            dt.bfloat16,
                            name=f"ring{w}_{chain}",
                            tag="ring",
                        )
                if tl == 0:
                    emit_x_dma(c + 2)
                for chain in range(S):
                    zt = z_tiles[(c, chain)]
                    # 4 V-matmuls accumulate on top of Z in PSUM
                    for h in (0, 1):
                        zcol = h * ZREG + tl * CB
                        for kh in (0, 1):
                            nc.tensor.matmul(
                                zt[:, zcol : zcol + CB],
                                w_slice(0, kh, h),
                                rhs_ap(t_now - 1, kh, chain),
                                start=False,
                                stop=(h == 1 and kh == 1 and tl == TCH - 1),
                                skip_group_check=True,
                            )
                    # tanh over both halves of this chain (AP across 2 banks)
                    zin = zt[:, :].rearrange("p (h t b) -> p h t b", h=2, t=TCH)[
                        :, :, tl, :
                    ]
                    hout = ring_view(t_now // OCH, chain)[:, :, t_now % OCH, :]
                    nc.scalar.activation(hout, zin, TANH)

                # interleave next chunk's Z work into PE idle windows,
                # one matmul per step so the in-order PE stream never
                # stalls the recurrence chain for long
                if tl >= 1 and pending_z:
                    pending_z.pop(0)()

                # output DMA at end of each OCH-step window
                if t_now % OCH == OCH - 1:
                    w = t_now // OCH
                    for chain in range(S):
                        src = ring_view(w, chain)
                        dst = out.ap()[chain].rearrange("h j t b -> j h t b")[
                            :, :, w * OCH : (w + 1) * OCH, :
                        ]
                        nc.sync.dma_start(out=dst, in_=src)

    nc.compile()
    return nc


def _get_module():
    if "nc" not in _built:
        _built["nc"] = _build_module()
    return _built["nc"]


def _prep_in_maps(encoder_inputs, decoder_inputs, U_i, V_i, b_i):
    enc = np.asarray(encoder_inputs, dtype=np.float32)
    dec = np.asarray(decoder_inputs, dtype=np.float32)
    U32 = np.asarray(U_i, dtype=np.float32)
    U = U32.astype(BF16)
    Ulo = (U32 - U.astype(np.float32)).astype(BF16)
    V = np.asarray(V_i, dtype=np.float32).astype(BF16)
    b = np.asarray(b_i, dtype=np.float32).reshape(1, HID).astype(BF16)
    ones = np.ones((1, 512), dtype=BF16)
    in_maps = []
    for c in range(NCORES):
        xs = dec[c * BC : (c + 1) * BC]                      # [BC, SEQ, IN]
        xT32 = np.ascontiguousarray(xs.transpose(2, 1, 0))   # [IN, SEQ, BC]
        xTc = xT32.astype(BF16)
        xTlo = (xT32 - xTc.astype(np.float32)).astype(BF16)
        xTc = xTc.reshape(2, 128, SEQ, BC)
        xTlo = xTlo.reshape(2, 128, SEQ, BC)
        h0c = enc[c * BC : (c + 1) * BC]                     # [BC, HID]
        h0T = np.ascontiguousarray(h0c.T).astype(BF16).reshape(2, 128, BC)
        in_maps.append(
            {
                "xT": xTc,
                "xTlo": xTlo,
                "V": V,
                "U": U,
                "Ulo": Ulo,
                "bvec": b,
                "ones": ones,
                "h0T": h0T,
            }
        )
    return in_maps


def kernel(encoder_inputs, decoder_inputs, U_i, V_i, b_i):
    global LAST_EXEC_NS
    from concourse import bass_utils

    nc = _get_module()
    in_maps = _prep_in_maps(encoder_inputs, decoder_inputs, U_i, V_i, b_i)
    trace = os.environ.get("KERNEL_TRACE", "0") == "1"
    res = bass_utils.run_bass_kernel_spmd(
        nc, in_maps, core_ids=list(range(NCORES)), trace=trace
    )
    LAST_EXEC_NS = res.exec_time_ns

    outs = []
    for c in range(NCORES):
        o = np.asarray(res.results[c]["out"]).astype(np.float32)  # [S,2,128,SEQ,CB]
        o = o.reshape(S, HID, SEQ, CB)          # j = 128*h + j'
        o = o.transpose(0, 3, 2, 1)             # [S, CB, SEQ, HID]
        outs.append(o.reshape(BC, SEQ, HID))    # b = chain*CB + b'
    dec_out = np.ascontiguousarray(np.concatenate(outs, axis=0))
    enc = np.asarray(encoder_inputs, dtype=np.float32)
    return (enc, dec_out)
